# revision 18
# baseline (speedup 1.0000x reference)
"""Trainium2 Bass kernel for nn_GATSTEMEncoder (2-layer GAT + Linear 1024->25088).

Self-contained: hardcodes all shapes; builds + compiles the Bass program on
first call (cached per graph structure) and runs it SPMD on 8 NeuronCores.

Design (v3):
- Nodes relabeled so core c owns new ids [c*1280,(c+1)*1280), degree-sorted
  within core. Edges live with their dst core as a slot-CSR (slot 0 = self
  loop). Per-edge gathers use dma_gather on a bf16 feature table
  T [10248, 1152] = [xw(1024) | al_src(4) | al_dst(4) | pad]; row c*1281+1280
  is a pad row with al_src = -1e9 so padded slots contribute exp()==0.
- Layer 1: every core computes the FULL xw1 table locally (cheap bf16 matmul)
  -> no AllGather and no AllReduce for layer 1.
- Layer 2: sharded xw2 (inlined into layer-1's edge loop per finished tile) +
  one bf16 AllGather + tiny AllReduce for the softmax stabilization bound.
- Per-edge attention logits from edge_features are HOST-precomputed.
- Edge aggregation accumulates on the Vector engine into SBUF f32 (pairwise
  bf16 pre-sums), keeping the Tensor engine free/idle-clocked work off the
  critical path.
- Final Linear row-sharded, n-chunks processed 4 at a time sharing each
  stationary hT block (ldweights reuse); PSUM is DMA'd straight to DRAM
  (bf==0 fast path).
"""
import os
import sys
import numpy as np
import ml_dtypes

for p in ("/opt/trn_rl_repo", "/root/.axon_site", "/root/.axon_site/_ro/trn_rl_repo"):
    if p not in sys.path:
        sys.path.append(p)

H, C = 4, 256
HC = H * C
N = 10000
NPAD = 10240
NCORES = 8
SHARD = NPAD // NCORES          # 1280
NT = SHARD // 128               # 10 tiles/core
SHARD1 = 1281                   # shard rows incl. inline pad row
TROWS = NCORES * SHARD1         # 10248 table rows
PAD_ROW = 1280                  # pad row of shard 0 (any shard works)
NEG = -1.0e9
D_IN = 128
E_DIM = 16
FCOLS = 1152                    # xw(1024)|al_src(4)|al_dst(4)|pad, bf16
                                # (gather elem bytes must be %256: 1152*2=2304)
NBLK = NPAD // 128              # 80 blocks for the full xw1 table
FOUT = 25088
G = 8                           # slot-chunks per dma_gather call
NCH = 512                       # final matmul N-chunk (25088 = 49*512)
NNCH = FOUT // NCH
NGF = 4                         # final n-chunks sharing one weight load
NPF = 10                        # final n-chunks interleaved into layer-2


# ----------------------------------------------------------------- host prep

def _fold_weights(W, a_src, a_dst):
    din = W.shape[0]
    Wr = W.reshape(din, H, C)
    W_ext = np.zeros((din, FCOLS), np.float32)
    W_ext[:, :HC] = W
    W_ext[:, HC:HC + H] = np.einsum('dhc,hc->dh', Wr, a_src)
    W_ext[:, HC + H:HC + 2 * H] = np.einsum('dhc,hc->dh', Wr, a_dst)
    return W_ext


def _fold_edge(We, a_edge):
    return np.einsum('dhc,hc->dh', We.reshape(E_DIM, H, C), a_edge).astype(np.float32)


def _build_shards(edge_index, edge_features, M1, M2):
    """Slot-CSR per dst core + host-precomputed per-slot edge-attn logits."""
    src = np.asarray(edge_index[0], np.int64)
    dst = np.asarray(edge_index[1], np.int64)

    order = np.argsort(dst, kind='stable')
    src_s = src[order]
    counts = np.bincount(dst[order], minlength=N)
    starts = np.concatenate([[0], np.cumsum(counts)])
    counts_pad = np.concatenate([counts, np.zeros(NPAD - N, np.int64)])

    perm = np.empty(NPAD, np.int64)
    for c in range(NCORES):
        lo = c * SHARD
        d = counts_pad[lo:lo + SHARD]
        perm[lo:lo + SHARD] = lo + np.argsort(-d, kind='stable')
    inv = np.empty(NPAD, np.int64)
    inv[perm] = np.arange(NPAD)
    deg_new = counts_pad[perm]

    KT = np.zeros(NT, np.int64)
    for t in range(NT):
        mx = 0
        for c in range(NCORES):
            d = deg_new[c * SHARD + t * 128: c * SHARD + (t + 1) * 128]
            mx = max(mx, int(d.max()))
        KT[t] = mx + 1
    S = int(KT.sum()) * 128

    # per-edge and per-node (loop) attention logits, original order
    alE_e = [edge_features @ M1, edge_features @ M2]        # [E,H] each
    loop_al = []
    for l in range(2):
        acc = np.zeros((N, H), np.float32)
        np.add.at(acc, dst, alE_e[l])
        loop_al.append(acc / np.maximum(counts, 1.0)[:, None])
    aeMax = np.stack([
        np.maximum(np.maximum(alE_e[0].max(0), loop_al[0].max(0)), 0.0),
        np.maximum(np.maximum(alE_e[1].max(0), loop_al[1].max(0)), 0.0)],
        axis=0).astype(np.float32)                     # [2,H]

    def remap(i):
        return (i // SHARD) * SHARD1 + (i % SHARD)

    shards = []
    for c in range(NCORES):
        src_slots = np.full(S, PAD_ROW, np.int64)
        alE_slots = np.zeros((2, S, H), np.float32)
        base = 0
        for t in range(NT):
            kt = int(KT[t])
            for p in range(128):
                nid_new = c * SHARD + t * 128 + p
                nid_old = perm[nid_new]
                if nid_old >= N:
                    continue
                src_slots[base + p] = remap(nid_new)
                alE_slots[0, base + p] = loop_al[0][nid_old]
                alE_slots[1, base + p] = loop_al[1][nid_old]
                d = int(counts_pad[nid_old])
                if d > 0:
                    e0 = starts[nid_old]
                    idxs = base + (np.arange(d) + 1) * 128 + p
                    src_slots[idxs] = remap(inv[src_s[e0:e0 + d]])
                    alE_slots[0, idxs] = alE_e[0][order[e0:e0 + d]]
                    alE_slots[1, idxs] = alE_e[1][order[e0:e0 + d]]
            base += kt * 128
        Ctot = S // 128
        alE_dev = alE_slots.reshape(2, Ctot, 128, H).transpose(0, 2, 1, 3).copy()
        shards.append((src_slots.astype(np.int32), alE_dev))
    return shards, KT, S, perm, inv, aeMax


# --------------------------------------------------------------- bass build

_CACHE = {}


def _build(KT, S, bzero, bfzero):
    import concourse.bass as bass
    import concourse.mybir as mybir
    import concourse.tile as tile
    from concourse import bacc
    from concourse.masks import make_identity

    f32 = mybir.dt.float32
    bf16 = mybir.dt.bfloat16
    i16 = mybir.dt.int16
    Ctot = S // 128
    NGRP = (Ctot + G - 1) // G
    KTmax = int(max(KT))
    chunk0 = np.concatenate([[0], np.cumsum(KT)]).astype(int)
    rg = [list(range(NCORES))]
    AF = mybir.ActivationFunctionType
    OP = mybir.AluOpType

    nc = bacc.Bacc("TRN2", target_bir_lowering=False, debug=False,
                   num_devices=NCORES)

    # -------- I/O
    xT_d = nc.dram_tensor("x_T", [D_IN, NPAD], bf16, kind="ExternalInput")
    W1e_d = nc.dram_tensor("W1e", [D_IN, FCOLS], bf16, kind="ExternalInput")
    W2e_d = nc.dram_tensor("W2e", [HC, FCOLS], bf16, kind="ExternalInput")
    alE1_d = nc.dram_tensor("alE1", [128, Ctot, H], f32, kind="ExternalInput")
    alE2_d = nc.dram_tensor("alE2", [128, Ctot, H], f32, kind="ExternalInput")
    mc_d = nc.dram_tensor("mconst", [1, 16], f32, kind="ExternalInput")
    bbc_d = nc.dram_tensor("b_bc", [128, 2, HC], f32, kind="ExternalInput")
    idx_d = nc.dram_tensor("idx16", [128, S // 16], i16, kind="ExternalInput")
    Wf_d = nc.dram_tensor("Wf", [HC, FOUT], bf16, kind="ExternalInput")
    bf_d = nc.dram_tensor("bf_bc", [128, FOUT], f32, kind="ExternalInput")
    out_d = nc.dram_tensor("out", [SHARD, FOUT], f32, kind="ExternalOutput")

    with tile.TileContext(nc) as tc:
        with (
            tc.tile_pool(name="const", bufs=1) as cpool,
            tc.tile_pool(name="dram", bufs=1, space="DRAM") as dpool,
            tc.tile_pool(name="persist", bufs=1) as ppool,
        ):
            # ---- constants
            ident = cpool.tile([128, 128], f32, tag="ident")
            make_identity(nc, ident[:])
            identb = cpool.tile([128, 128], bf16, tag="identb")
            make_identity(nc, identb[:])
            identb3 = cpool.tile([128, 1, 128], bf16, tag="identb3")
            make_identity(nc, identb3[:, 0, :])
            ones_row = cpool.tile([1, 128], f32, tag="ones_row")
            nc.vector.memset(ones_row[:], 1.0)
            padrow = cpool.tile([1, FCOLS], bf16, tag="padrow")
            nc.vector.memset(padrow[:], 0.0)
            nc.vector.memset(padrow[:, HC:HC + H], NEG)
            mc_sb = cpool.tile([1, 16], f32, tag="mc")
            nc.sync.dma_start(out=mc_sb[:], in_=mc_d.ap())
            idx_sb = cpool.tile([128, S // 16], i16, tag="idx")
            nc.sync.dma_start(out=idx_sb[:], in_=idx_d.ap())

            # persistent strips (h^T), reused layer1 -> layer2
            hT = [ppool.tile([128, SHARD], bf16, tag=f"hT{k}", name=f"hT{k}")
                  for k in range(8)]
            nm = [ppool.tile([128, 2 * H], f32, tag=f"nm{l}", name=f"nm{l}")
                  for l in range(2)]
            mnegc = [ppool.tile([128, 1], f32, tag=f"mnegc{l}", name=f"mnegc{l}")
                     for l in range(2)]

            # DRAM tables
            T1 = dpool.tile([TROWS, FCOLS], bf16, tag="T1", name="T1")
            Tloc2 = dpool.tile([SHARD1, FCOLS], bf16, tag="Tloc2", name="Tloc2")
            Tfull2 = dpool.tile([TROWS, FCOLS], bf16, tag="Tfull2", name="Tfull2",
                                addr_space="Shared")
            mx_in = dpool.tile([1, 8], f32, tag="mx_in", name="mx_in")
            mx_out = dpool.tile([1, 8], f32, tag="mx_out", name="mx_out",
                                addr_space="Shared")

            nc.vector.memset(nm[0][:], NEG)
            nc.vector.memset(nm[1][:], NEG)

            # ---------------- helper: exp-bound -> mnegc[L]
            def make_bound(L, allreduce, mpool, mpsum):
                trp = mpsum.tile([8, 128], f32, tag="btr")
                nc.tensor.transpose(trp[:], nm[L][:], ident[:])
                mx8 = mpool.tile([8, 128], f32, tag="mx8")
                nc.vector.tensor_copy(mx8[:], trp[:])
                mxc = mpool.tile([8, 1], f32, tag="mxc")
                nc.vector.reduce_max(out=mxc[:], in_=mx8[:],
                                     axis=mybir.AxisListType.X)
                nc.sync.dma_start(out=mx_in[:].rearrange("a b -> b a"),
                                  in_=mxc[:])
                if allreduce:
                    nc.gpsimd.collective_compute(
                        "AllReduce", OP.max, replica_groups=rg,
                        ins=[mx_in[:]], outs=[mx_out[:]])
                    src_t = mx_out
                else:
                    src_t = mx_in
                mrow = mpool.tile([1, 8], f32, tag="mrow")
                nc.sync.dma_start(out=mrow[:], in_=src_t[:])
                t4 = mpool.tile([1, 4], f32, tag="t4")
                nc.vector.tensor_add(t4[:], mrow[:, 0:4], mrow[:, 4:8])
                nc.vector.tensor_tensor(out=t4[:], in0=t4[:],
                                        in1=mc_sb[:, L * 4:L * 4 + 4], op=OP.add)
                nc.vector.tensor_scalar_max(t4[:], t4[:], 0.0)
                mneg = mpool.tile([1, 1], f32, tag="mneg")
                nc.vector.reduce_max(out=mneg[:], in_=t4[:],
                                     axis=mybir.AxisListType.X)
                nc.vector.tensor_scalar_mul(mneg[:], mneg[:], -1.0)
                bps = mpsum.tile([128, 1], f32, tag="bps")
                nc.tensor.matmul(bps[:], ones_row[:], mneg[:],
                                 start=True, stop=True)
                nc.vector.tensor_copy(mnegc[L][:], bps[:])

            # ---------------- edge loop (shared for both layers)
            def edge_layer(L, table, alE_t, on_tile_done, bbc, nb=2):
                with (
                    tc.tile_pool(name=f"gp{L}", bufs=nb) as gp,
                    tc.tile_pool(name=f"mp{L}", bufs=nb) as mp,
                    tc.tile_pool(name=f"sp{L}", bufs=4) as sp,
                    tc.tile_pool(name=f"st{L}", bufs=2) as stp,
                    tc.tile_pool(name=f"ep{L}", bufs=nb) as ep,
                    tc.tile_pool(name=f"ag{L}", bufs=2, space="PSUM") as agp,
                    tc.tile_pool(name=f"tr{L}", bufs=2, space="PSUM") as trp,
                ):
                    state = {}
                    for grp in range(NGRP):
                        cb0 = grp * G
                        gl = min(G, Ctot - cb0)
                        g_sb = gp.tile([128, G, FCOLS], bf16, tag="g_sb")
                        nc.gpsimd.dma_gather(
                            out_ap=g_sb[:, 0:gl, :], in_ap=table[:],
                            idxs_ap=idx_sb[:, cb0 * 8:(cb0 + gl) * 8],
                            num_idxs=gl * 128, num_idxs_reg=gl * 128,
                            elem_size=FCOLS)
                        exp_bf = sp.tile([128, G, H], bf16, tag="exp_bf")
                        segs = []
                        for t in range(NT):
                            a = max(cb0, int(chunk0[t]))
                            b = min(cb0 + gl, int(chunk0[t + 1]))
                            if a < b:
                                segs.append((t, a, b))
                        for (t, ca, cb) in segs:
                            c0 = int(chunk0[t])
                            if ca == c0:
                                st = {
                                    'psum': agp.tile([128, HC], f32, tag="agg",
                                                     name="agg"),
                                    'expt': stp.tile([128, H, KTmax], f32,
                                                     tag="expt", name="expt"),
                                    'ald': stp.tile([128, 1, H], f32,
                                                    tag="ald", name="ald"),
                                }
                                state[t] = st
                                nc.vector.tensor_copy(
                                    st['ald'][:, 0, :],
                                    g_sb[:, ca - cb0, HC + H:HC + 2 * H])
                            st = state[t]
                            n = cb - ca
                            ga = ca - cb0
                            l0 = sp.tile([128, G, H], f32, tag="l0")
                            nc.vector.tensor_tensor(
                                out=l0[:, 0:n], in0=g_sb[:, ga:ga + n, HC:HC + H],
                                in1=alE_t[:, ca:cb, :], op=OP.add)
                            nc.vector.tensor_tensor(
                                out=l0[:, 0:n], in0=l0[:, 0:n],
                                in1=st['ald'][:].to_broadcast([128, n, H]),
                                op=OP.add)
                            nc.vector.scalar_tensor_tensor(
                                out=l0[:, 0:n], in0=l0[:, 0:n], scalar=0.2,
                                in1=l0[:, 0:n], op0=OP.mult, op1=OP.max)
                            k0 = ca - c0
                            nc.scalar.activation(
                                out=st['expt'][:, :, k0:k0 + n],
                                in_=l0[:, 0:n].rearrange("p g h -> p h g"),
                                func=AF.Exp, bias=mnegc[L][:], scale=1.0)
                            nc.scalar.activation(
                                out=exp_bf[:, ga:ga + n, :],
                                in_=l0[:, 0:n],
                                func=AF.Exp, bias=mnegc[L][:], scale=1.0)
                        scaled = mp.tile([128, G, H, C], bf16, tag="scaled")
                        nc.vector.tensor_tensor(
                            out=scaled[:, 0:gl],
                            in0=g_sb[:, 0:gl, 0:HC]
                            .rearrange("p g (h c) -> p g h c", h=H),
                            in1=exp_bf[:, 0:gl].to_broadcast([128, gl, H, C]),
                            op=OP.mult)
                        sc2 = scaled[:].rearrange("p g h c -> p (g h c)")
                        for (t, ca, cb) in segs:
                            st = state[t]
                            c0, c1 = int(chunk0[t]), int(chunk0[t + 1])
                            for c in range(ca, cb):
                                g = c - cb0
                                nc.tensor.matmul(
                                    st['psum'][:, 0:512], identb[:],
                                    sc2[:, g * HC:g * HC + 512],
                                    start=(c == c0), stop=(c == c1 - 1))
                                nc.tensor.matmul(
                                    st['psum'][:, 512:1024], identb[:],
                                    sc2[:, g * HC + 512:(g + 1) * HC],
                                    start=(c == c0), stop=(c == c1 - 1))
                            if cb == c1:
                                # ---- tile epilogue
                                kt = c1 - c0
                                s_t = ep.tile([128, H], f32, tag="s_t")
                                nc.vector.reduce_sum(
                                    out=s_t[:], in_=st['expt'][:, :, 0:kt],
                                    axis=mybir.AxisListType.X)
                                nc.vector.tensor_scalar_add(s_t[:], s_t[:], 1e-16)
                                rec = ep.tile([128, H], f32, tag="rec")
                                nc.vector.reciprocal(rec[:], s_t[:])
                                h_sb = ep.tile([128, HC], f32, tag="h_sb")
                                for h in range(H):
                                    nc.scalar.activation(
                                        out=h_sb[:, h * C:(h + 1) * C],
                                        in_=st['psum'][:, h * C:(h + 1) * C],
                                        func=AF.Copy, scale=rec[:, h:h + 1])
                                if not bzero:
                                    nc.vector.tensor_tensor(
                                        out=h_sb[:], in0=h_sb[:],
                                        in1=bbc[:, L, :], op=OP.add)
                                u = ep.tile([128, HC], f32, tag="u")
                                nc.vector.tensor_scalar_min(u[:], h_sb[:], 0.0)
                                nc.scalar.activation(out=u[:], in_=u[:],
                                                     func=AF.Exp)
                                h2 = ep.tile([128, HC], bf16, tag="h2")
                                nc.vector.scalar_tensor_tensor(
                                    out=h2[:], in0=u[:], scalar=-1.0,
                                    in1=h_sb[:], op0=OP.add, op1=OP.max)
                                for k8 in range(8):
                                    tp = trp.tile([128, 128], bf16, tag="tp")
                                    nc.tensor.transpose(
                                        tp[:], h2[:, k8 * 128:(k8 + 1) * 128],
                                        identb[:])
                                    nc.vector.tensor_copy(
                                        hT[k8][:, t * 128:(t + 1) * 128], tp[:])
                                del state[t]
                                if on_tile_done is not None:
                                    on_tile_done(t)

            # ---------------- phase X1: full xw1 table on every core
            with (
                tc.tile_pool(name="x1c", bufs=1) as x1c,
                tc.tile_pool(name="x1w", bufs=3) as wp,
                tc.tile_pool(name="x1p", bufs=3, space="PSUM") as pp,
                tc.tile_pool(name="x1p2", bufs=2, space="PSUM") as pp2,
            ):
                xT_sb = x1c.tile([D_IN, NPAD], bf16, tag="xT")
                nc.sync.dma_start(out=xT_sb[:], in_=xT_d.ap())
                W1e_sb = x1c.tile([D_IN, FCOLS], bf16, tag="W1e")
                nc.sync.dma_start(out=W1e_sb[:], in_=W1e_d.ap())
                for m in range(NBLK):
                    lt = xT_sb[:, m * 128:(m + 1) * 128]
                    ps0 = pp.tile([128, 512], f32, tag="ps0")
                    nc.tensor.matmul(ps0[:], lt, W1e_sb[:, 0:512],
                                     start=True, stop=True)
                    ps1 = pp.tile([128, 512], f32, tag="ps1")
                    nc.tensor.matmul(ps1[:], lt, W1e_sb[:, 512:1024],
                                     start=True, stop=True)
                    ps2 = pp2.tile([128, 8], f32, tag="ps2")
                    nc.tensor.matmul(ps2[:], lt, W1e_sb[:, 1024:1032],
                                     start=True, stop=True)
                    sb = wp.tile([128, FCOLS], bf16, tag="xsb")
                    nc.vector.tensor_copy(sb[:, 0:512], ps0[:])
                    nc.scalar.activation(out=sb[:, 512:1024], in_=ps1[:],
                                         func=AF.Copy)
                    nc.vector.tensor_copy(sb[:, 1024:1032], ps2[:])
                    nc.vector.tensor_tensor(out=nm[0][:], in0=nm[0][:],
                                            in1=ps2[:], op=OP.max)
                    r0 = (m // NT) * SHARD1 + (m % NT) * 128
                    nc.sync.dma_start(out=T1[r0:r0 + 128, :], in_=sb[:])
                for c in range(NCORES):
                    nc.sync.dma_start(
                        out=T1[c * SHARD1 + SHARD:c * SHARD1 + SHARD1, :],
                        in_=padrow[:])
            with (
                tc.tile_pool(name="b1", bufs=1) as b1p,
                tc.tile_pool(name="b1ps", bufs=1, space="PSUM") as b1ps,
            ):
                make_bound(0, False, b1p, b1ps)

            # ---------------- layers (alE/bbc scoped to the edge phases)
            with tc.tile_pool(name="econst", bufs=1) as ecp:
                alE2_sb = ecp.tile([128, Ctot, H], f32, tag="alE1t")
                nc.sync.dma_start(out=alE2_sb[:], in_=alE2_d.ap())
                if bzero:
                    bbc = None
                else:
                    bbc = ecp.tile([128, 2, HC], f32, tag="bbc")
                    nc.sync.dma_start(out=bbc[:], in_=bbc_d.ap())

                # layer 1 with xw2 inlined per finished tile
                with (
                    tc.tile_pool(name="w2c", bufs=1) as w2c,
                    tc.tile_pool(name="x2w", bufs=2) as w2p,
                    tc.tile_pool(name="x2p", bufs=1, space="PSUM") as x2pp,
                ):
                    alE1_sb = w2c.tile([128, Ctot, H], f32, tag="alE0t")
                    nc.sync.dma_start(out=alE1_sb[:], in_=alE1_d.ap())
                    W2e_sb = w2c.tile([128, 8, FCOLS], bf16, tag="W2e")
                    nc.sync.dma_start(
                        out=W2e_sb[:],
                        in_=W2e_d.ap().rearrange("(ko p) n -> p ko n", p=128))

                    def xw2_block(t):
                        sb = w2p.tile([128, FCOLS], bf16, tag="t2sb")
                        for (n0, nn, tg) in ((0, 512, "xq0"), (512, 512, "xq0"),
                                             (1024, 8, "xq2")):
                            ps = x2pp.tile([128, nn], f32, tag=tg, name=tg)
                            for k in range(8):
                                nc.tensor.matmul(
                                    ps[:, 0:nn],
                                    hT[k][:, t * 128:(t + 1) * 128],
                                    W2e_sb[:, k, n0:n0 + nn],
                                    start=(k == 0), stop=(k == 7))
                            if n0 == 1024:
                                nc.vector.tensor_tensor(
                                    out=nm[1][:], in0=nm[1][:], in1=ps[:],
                                    op=OP.max)
                                nc.vector.tensor_copy(sb[:, n0:n0 + nn], ps[:])
                            elif n0 == 0:
                                nc.vector.tensor_copy(sb[:, n0:n0 + nn], ps[:])
                            else:
                                nc.scalar.activation(out=sb[:, n0:n0 + nn],
                                                     in_=ps[:], func=AF.Copy)
                        nc.sync.dma_start(out=Tloc2[t * 128:(t + 1) * 128, :],
                                          in_=sb[:])

                    edge_layer(0, T1, alE1_sb, xw2_block, bbc, nb=3)
                    nc.sync.dma_start(out=Tloc2[SHARD:SHARD1, :], in_=padrow[:])

                # bound for layer 2 (AllReduce) + AllGather of the table
                with (
                    tc.tile_pool(name="b2", bufs=1) as b2p,
                    tc.tile_pool(name="b2ps", bufs=1, space="PSUM") as b2ps,
                ):
                    make_bound(1, True, b2p, b2ps)
                nc.gpsimd.collective_compute(
                    "AllGather", OP.bypass, replica_groups=rg,
                    ins=[Tloc2[:]], outs=[Tfull2[:]])

                # layer 2, with the first NPF final-Linear n-chunks interleaved
                # per finished tile (Wf prefetched during the AllGather window)
                with (
                    tc.tile_pool(name="pfw", bufs=1) as pfw_p,
                    tc.tile_pool(name="fse", bufs=2) as fse,
                    tc.tile_pool(name="fpe", bufs=2, space="PSUM") as fpe,
                ):
                    pfw_sb = pfw_p.tile([128, 8, NPF * NCH], bf16, tag="pfw")
                    nc.sync.dma_start(
                        out=pfw_sb[:],
                        in_=Wf_d.ap()[:, 0:NPF * NCH]
                        .rearrange("(ko p) n -> p ko n", p=128))
                    if not bfzero:
                        pfb_sb = pfw_p.tile([128, NPF * NCH], f32, tag="pfb")
                        nc.sync.dma_start(out=pfb_sb[:],
                                          in_=bf_d.ap()[:, 0:NPF * NCH])

                    def fin_tile(t):
                        for j in range(NPF):
                            ps = fpe.tile([128, NCH], f32, tag="fpe")
                            for k in range(8):
                                nc.tensor.matmul(
                                    ps[:], hT[k][:, t * 128:(t + 1) * 128],
                                    pfw_sb[:, k, j * NCH:(j + 1) * NCH],
                                    start=(k == 0), stop=(k == 7))
                            strip = fse.tile([128, NCH], f32, tag="fse")
                            if bfzero:
                                nc.vector.tensor_copy(strip[:], ps[:])
                            else:
                                nc.vector.tensor_add(
                                    strip[:], ps[:],
                                    pfb_sb[:, j * NCH:(j + 1) * NCH])
                            nc.sync.dma_start(
                                out=out_d.ap()[t * 128:(t + 1) * 128,
                                               j * NCH:(j + 1) * NCH],
                                in_=strip[:])

                    edge_layer(1, Tfull2, alE2_sb, fin_tile, bbc)

            # ---------------- final row-sharded Linear: out = h2 @ Wf + bf
            with (
                tc.tile_pool(name="fin", bufs=2) as fp,
                tc.tile_pool(name="finb", bufs=1) as fb,
                tc.tile_pool(name="fstr", bufs=2) as fstr,
                tc.tile_pool(name="finps", bufs=2, space="PSUM") as fpp,
            ):
                n = NPF
                while n < NNCH:
                    ln = min(NGF, NNCH - n)
                    n0 = n * NCH
                    wf_sb = fp.tile([128, 8, NGF * NCH], bf16, tag="wf_sb")
                    nc.sync.dma_start(
                        out=wf_sb[:, :, 0:ln * NCH],
                        in_=Wf_d.ap()[:, n0:n0 + ln * NCH]
                        .rearrange("(ko p) n -> p ko n", p=128))
                    if not bfzero:
                        bf_sb = fb.tile([128, NGF * NCH], f32, tag="bf_sb")
                        nc.sync.dma_start(out=bf_sb[:, 0:ln * NCH],
                                          in_=bf_d.ap()[:, n0:n0 + ln * NCH])
                    for m in range(NT):
                        pss = [fpp.tile([128, NCH], f32, tag=f"fp{j}",
                                        name=f"fp{j}") for j in range(ln)]
                        for k in range(8):
                            for j in range(ln):
                                mm = nc.tensor.matmul(
                                    pss[j][:], hT[k][:, m * 128:(m + 1) * 128],
                                    wf_sb[:, k, j * NCH:(j + 1) * NCH],
                                    start=(k == 0), stop=(k == 7),
                                    skip_group_check=True)
                                if j > 0:
                                    mm.ins.ldweights = False
                        for j in range(ln):
                            strip = fstr.tile([128, NCH], f32,
                                              tag=f"str{j}", name=f"str{j}")
                            if bfzero:
                                nc.vector.tensor_copy(strip[:], pss[j][:])
                            else:
                                nc.vector.tensor_add(
                                    strip[:], pss[j][:],
                                    bf_sb[:, j * NCH:(j + 1) * NCH])
                            nc.sync.dma_start(
                                out=out_d.ap()[m * 128:(m + 1) * 128,
                                               n0 + j * NCH:n0 + (j + 1) * NCH],
                                in_=strip[:])
                    n += ln

    nc.compile()
    return nc


# ------------------------------------------------------------------- driver

def kernel(**inputs):
    from concourse.bass_utils import run_bass_kernel_spmd

    x = np.asarray(inputs["x"], np.float32)
    ei = np.asarray(inputs["edge_index"])
    ef = np.asarray(inputs["edge_features"], np.float32)

    M1 = _fold_edge(np.asarray(inputs["We1"], np.float32),
                    np.asarray(inputs["att_edge1"], np.float32))
    M2 = _fold_edge(np.asarray(inputs["We2"], np.float32),
                    np.asarray(inputs["att_edge2"], np.float32))
    shards, KT, S, perm, inv, aeMax = _build_shards(ei, ef, M1, M2)
    bzero = not (np.any(np.asarray(inputs["b1"])) or
                 np.any(np.asarray(inputs["b2"])))
    bfzero = not np.any(np.asarray(inputs["bf"]))
    key = (S, tuple(int(k) for k in KT), bzero, bfzero)
    if key not in _CACHE:
        _CACHE[key] = _build(KT, S, bzero, bfzero)
    nc = _CACHE[key]

    W1e = _fold_weights(np.asarray(inputs["W1"], np.float32),
                        np.asarray(inputs["att_src1"], np.float32),
                        np.asarray(inputs["att_dst1"], np.float32)
                        ).astype(ml_dtypes.bfloat16)
    W2e = _fold_weights(np.asarray(inputs["W2"], np.float32),
                        np.asarray(inputs["att_src2"], np.float32),
                        np.asarray(inputs["att_dst2"], np.float32)
                        ).astype(ml_dtypes.bfloat16)
    mconst = np.zeros((1, 16), np.float32)
    mconst[0, 0:4] = aeMax[0]
    mconst[0, 4:8] = aeMax[1]
    bbc = np.broadcast_to(
        np.stack([np.asarray(inputs["b1"], np.float32),
                  np.asarray(inputs["b2"], np.float32)])[None],
        (128, 2, HC)).copy()
    Wf = np.ascontiguousarray(
        np.asarray(inputs["Wf"], np.float32).astype(ml_dtypes.bfloat16))
    bfbc = np.broadcast_to(np.asarray(inputs["bf"], np.float32)[None],
                           (128, FOUT)).copy()

    xpad = np.zeros((NPAD, D_IN), np.float32)
    xpad[:N] = x
    x_new = xpad[np.where(perm < N, perm, 0)]
    x_new[perm >= N] = 0.0
    xT = np.ascontiguousarray(x_new.T).astype(ml_dtypes.bfloat16)

    in_maps = []
    for c in range(NCORES):
        src_slots, alE_dev = shards[c]
        idx16 = np.tile(src_slots.astype(np.int16).reshape(S // 16, 16).T,
                        (8, 1)).copy()
        in_maps.append({
            "x_T": xT, "W1e": W1e, "W2e": W2e,
            "alE1": np.ascontiguousarray(alE_dev[0]),
            "alE2": np.ascontiguousarray(alE_dev[1]),
            "mconst": mconst, "b_bc": bbc,
            "idx16": idx16,
            "Wf": Wf, "bf_bc": bfbc,
        })

    trace = os.environ.get("KERNEL_TRACE", "") == "1"
    res = run_bass_kernel_spmd(nc, in_maps, core_ids=list(range(NCORES)),
                               trace=trace,
                               trace_cores=[0] if trace else None)
    global _last_results
    _last_results = res
    out_new = np.concatenate([res.results[c]["out"] for c in range(NCORES)],
                             axis=0)          # [NPAD, FOUT] in new node order
    return out_new[inv[:N]]


_last_results = None
